# revision 19
# baseline (speedup 1.0000x reference)
"""Trainium2 Bass kernel for nn_NodeSearch (GNN message passing, G=32 graphs).

Sharding: data-parallel over graphs — 8 cores x 4 graphs each. Each core:
  for each of its graphs:
    deg = row-sums of A            (DVE reduce over A-natural slabs)
    T = A^T resident in SBUF       (PE tile transposes; PE contracts over
                                    partitions, so A@x products need A^T)
    u = (x @ W_gcn) / deg[:,None]
    y1^T = (A @ u)^T               (PE, u blocks stationary, T streaming)
    hn = elu(y1 + b_gcn); LN stats; scores = hn_nat . p_select
    top-64 of scores               (per-partition max8 -> 1024 candidates ->
                                    global rank via PE broadcast + DVE counts)
    v = (hn - mu)*inv/deg[:,None]
    Z = (A @ v)^T -> agg natural   (PE transposes back)
    sub_reps^T = agg^T @ Sel       (Sel one-hot built from sorted indices)
    conv1d(k=2) over rank dim + relu + maxpool + alpha-mix -> B[g] in [64]
Per-core output: B rows [4, 64]. The tiny discriminator tail (roll, original @
W_disc, dot products -> [64]) is computed on host from the gathered B.
"""

import os
import numpy as np

import tile_patch

tile_patch.apply()

import concourse.bass as bass
import concourse.bass2jax as _b2j
import concourse.mybir as mybir
from concourse.bass_utils import run_bass_kernel_spmd
from concourse.masks import make_identity
from concourse.tile import TileContext

# Disk-cache compiled NEFFs keyed on the BIR hash — walrus compiles of this
# kernel take minutes and the BIR is deterministic for a given build.
_NEFF_CACHE = "/tmp/neff_cache"
_orig_compile_bir = _b2j.compile_bir_kernel


def _cached_compile_bir(ant_bir_str, compile_dir_path, neff_name="file.neff"):
    import hashlib
    import shutil

    h = hashlib.sha256(ant_bir_str).hexdigest()[:24]
    cpath = os.path.join(_NEFF_CACHE, f"{h}_{neff_name}")
    dst = os.path.join(compile_dir_path, neff_name)
    if os.path.exists(cpath):
        shutil.copy(cpath, dst)
        return dst
    out = _orig_compile_bir(ant_bir_str, compile_dir_path, neff_name=neff_name)
    os.makedirs(_NEFF_CACHE, exist_ok=True)
    shutil.copy(out, cpath)
    return out


_b2j.compile_bir_kernel = _cached_compile_bir

f32 = mybir.dt.float32
i32 = mybir.dt.int32
u32 = mybir.dt.uint32
Alu = mybir.AluOpType
Act = mybir.ActivationFunctionType
AX = mybir.AxisListType

G, N, D, L, K = 32, 2048, 128, 64, 64
NCORES = 8
GP = G // NCORES          # graphs per core
NB = N // 128             # 16 node blocks
LN_EPS = 1e-5


def build_nc(n_graphs=GP):
    nc = bass.Bass()
    g4 = nc.dram_tensor("g4", [n_graphs, N, N], f32, kind="ExternalInput")
    x4 = nc.dram_tensor("x4", [n_graphs, N, D], f32, kind="ExternalInput")
    wg = nc.dram_tensor("w_gcn", [D, L], f32, kind="ExternalInput")
    bg = nc.dram_tensor("b_gcn", [L], f32, kind="ExternalInput")
    ps = nc.dram_tensor("p_select", [L], f32, kind="ExternalInput")
    ck = nc.dram_tensor("conv_k", [2, L, L], f32, kind="ExternalInput")
    cb = nc.dram_tensor("conv_b", [L], f32, kind="ExternalInput")
    la = nc.dram_tensor("log_alphas", [1, 3], f32, kind="ExternalInput")
    b_out = nc.dram_tensor("b_out", [n_graphs, L], f32, kind="ExternalOutput")

    with TileContext(nc) as tc:
        with (
            tc.tile_pool(name="const", bufs=1) as pc,
            tc.tile_pool(name="pergraph", bufs=1) as pg,
            tc.tile_pool(name="slabs", bufs=2) as psl,
            tc.tile_pool(name="chunks", bufs=3) as pch,
            tc.tile_pool(name="dram", bufs=1, space="DRAM") as pdr,
            tc.tile_pool(name="ps_big", bufs=1, space="PSUM") as ppb,
            tc.tile_pool(name="ps_t", bufs=2, space="PSUM") as ppt,
            tc.tile_pool(name="ps_bc", bufs=1, space="PSUM") as ppbc,
        ):
            # ---- constants ----
            ident = pc.tile([128, 128], f32, tag="ident")
            make_identity(nc, ident[:, :])
            ones_row = pc.tile([1, 128], f32, tag="ones_row")
            nc.vector.memset(ones_row[:, :], 1.0)
            ones64 = pc.tile([64, 1], f32, tag="ones64")
            nc.vector.memset(ones64[:, :], 1.0)

            iota_i = pc.tile([128, 1], i32, tag="iota_i")
            nc.gpsimd.iota(iota_i[:, :], pattern=[[0, 1]], base=0, channel_multiplier=1)
            iotacolf = pc.tile([128, 1], f32, tag="iotacolf")
            nc.vector.tensor_copy(iotacolf[:, :], iota_i[:, :])

            mval_i = pc.tile([128, NB], i32, tag="mval_i")
            nc.gpsimd.iota(mval_i[:, :], pattern=[[128, NB]], base=0, channel_multiplier=1)
            mvalcols = pc.tile([128, NB], f32, tag="mvalcols")
            nc.vector.tensor_copy(mvalcols[:, :], mval_i[:, :])

            i64_i = pc.tile([128, 64], i32, tag="i64_i")
            nc.gpsimd.iota(i64_i[:, :], pattern=[[1, 64]], base=0, channel_multiplier=0)
            iota64bc = pc.tile([128, 64], f32, tag="iota64bc")
            nc.vector.tensor_copy(iota64bc[:, :], i64_i[:, :])

            wg_sb = pc.tile([D, L], f32, tag="wg")
            nc.sync.dma_start(wg_sb[:, :], wg[:, :])
            bg_col = pc.tile([L, 1], f32, tag="bg")
            nc.sync.dma_start(bg_col[:, :], bg[:].unsqueeze(-1))
            p_bc = pc.tile([128, L], f32, tag="p_bc")
            nc.sync.dma_start(p_bc[:, :], ps[:].unsqueeze(0).to_broadcast([128, L]))
            k0_sb = pc.tile([L, L], f32, tag="k0")
            nc.sync.dma_start(k0_sb[:, :], ck[0])
            k1_sb = pc.tile([L, L], f32, tag="k1")
            nc.sync.dma_start(k1_sb[:, :], ck[1])
            cb_col = pc.tile([L, 1], f32, tag="cb")
            nc.sync.dma_start(cb_col[:, :], cb[:].unsqueeze(-1))

            # alphas = softmax(log_alphas)
            la_sb = pc.tile([1, 3], f32, tag="la")
            nc.sync.dma_start(la_sb[:, :], la[:, :])
            ea = pc.tile([1, 3], f32, tag="ea")
            nc.scalar.activation(ea[:, :], la_sb[:, :], Act.Exp)
            easum = pc.tile([1, 1], f32, tag="easum")
            nc.vector.tensor_reduce(easum[:, :], ea[:, :], axis=AX.X, op=Alu.add)
            earec = pc.tile([1, 1], f32, tag="earec")
            nc.vector.reciprocal(earec[:, :], easum[:, :])
            alph_row = pc.tile([1, 3], f32, tag="alph")
            nc.vector.tensor_scalar_mul(alph_row[:, :], ea[:, :], earec[:, :])
            # broadcast alphas to 64 partitions via K=1 matmul
            ab_ps = ppt.tile([128, 512], f32, tag="pt")
            nc.tensor.matmul(ab_ps[:64, 0:3], ones_row[:, 0:64], alph_row[:, :],
                             start=True, stop=True)
            alph_bc = pc.tile([64, 3], f32, tag="alph_bc")
            nc.any.tensor_copy(alph_bc[:, :], ab_ps[:64, 0:3])

            for gi in range(n_graphs):
                # ---- persistent per-graph tiles ----
                T = pg.tile([128, NB * N], f32, tag="T")
                degcol = pg.tile([128, NB], f32, tag="degcol")
                unat = pg.tile([128, NB * L], f32, tag="unat")
                hnnat = pg.tile([128, NB * L], f32, tag="hnnat")
                vnat = pg.tile([128, NB * L], f32, tag="vnat")
                aggnat = pg.tile([128, NB * L], f32, tag="aggnat")
                scorecol = pg.tile([128, NB], f32, tag="scorecol")
                spart = pg.tile([64, NB], f32, tag="spart")
                sqpart = pg.tile([64, NB], f32, tag="sqpart")

                # ---- load A slabs: deg + transposes into T ----
                for mb in range(NB):
                    slab = psl.tile([128, N], f32, tag="slab")
                    nc.sync.dma_start(slab[:, :], g4[gi, mb * 128:(mb + 1) * 128, :])
                    nc.vector.tensor_reduce(
                        degcol[:, mb:mb + 1], slab[:, :], axis=AX.X, op=Alu.add
                    )
                    for kb in range(NB):
                        tps = ppt.tile([128, 512], f32, tag="pt")
                        nc.tensor.transpose(
                            tps[:, 0:128], slab[:, kb * 128:(kb + 1) * 128], ident[:, :]
                        )
                        nc.any.tensor_copy(
                            T[:, kb * N + mb * 128: kb * N + (mb + 1) * 128],
                            tps[:, 0:128],
                        )

                rdeg = pg.tile([128, NB], f32, tag="rdeg")
                nc.vector.reciprocal(rdeg[:, :], degcol[:, :])

                # ---- x load + transpose + xw = x @ W, u = xw * rdeg ----
                xnat = pg.tile([128, NB * D], f32, tag="xnat")
                nc.sync.dma_start(
                    xnat.rearrange("p (b d) -> p b d", b=NB),
                    x4[gi].rearrange("(b p) d -> p b d", p=128),
                )
                xT = pg.tile([128, NB * D], f32, tag="xT")
                for b in range(NB):
                    tps = ppt.tile([128, 512], f32, tag="pt")
                    nc.tensor.transpose(
                        tps[:, 0:128], xnat[:, b * D:(b + 1) * D], ident[:, :]
                    )
                    nc.any.tensor_copy(xT[:, b * D:(b + 1) * D], tps[:, 0:128])
                for b in range(NB):
                    tps = ppt.tile([128, 512], f32, tag="pt")
                    nc.tensor.matmul(
                        tps[:, 0:L], xT[:, b * D:(b + 1) * D], wg_sb[:, :],
                        start=True, stop=True,
                    )
                    nc.vector.tensor_scalar_mul(
                        unat[:, b * L:(b + 1) * L], tps[:, 0:L], rdeg[:, b:b + 1]
                    )

                # ---- mm1: y1T = (A @ u)^T  [64, N] fp32 ----
                y1t = ppb.tile([64, N], f32, tag="big")
                for kb in range(NB):
                    for c in range(N // 512):
                        nc.tensor.matmul(
                            y1t[:, c * 512:(c + 1) * 512],
                            unat[:, kb * L:(kb + 1) * L],
                            T[:, kb * N + c * 512: kb * N + (c + 1) * 512],
                            start=(kb == 0), stop=(kb == NB - 1),
                        )

                # ---- elu + stats + hn natural + scores ----
                for b in range(NB):
                    ysl = y1t[:, b * 128:(b + 1) * 128]
                    hc = pch.tile([64, 128], f32, tag="hc")
                    tch = pch.tile([64, 128], f32, tag="tch")
                    # relu(y + b) and expm1(min(y + b, 0))
                    nc.scalar.activation(hc[:, :], ysl, Act.Relu, bias=bg_col[:, :])
                    nc.vector.tensor_scalar(
                        tch[:, :], ysl, bg_col[:, :], 0.0, op0=Alu.add, op1=Alu.min
                    )
                    nc.scalar.activation(tch[:, :], tch[:, :], Act.Exp)
                    nc.vector.tensor_add(hc[:, :], hc[:, :], tch[:, :])
                    nc.vector.tensor_scalar_add(hc[:, :], hc[:, :], -1.0)
                    # stats partials
                    nc.vector.tensor_reduce(
                        spart[:, b:b + 1], hc[:, :], axis=AX.X, op=Alu.add
                    )
                    nc.scalar.activation(
                        tch[:, :], hc[:, :], Act.Square, accum_out=sqpart[:, b:b + 1]
                    )
                    # hn natural block
                    tps = ppt.tile([128, 512], f32, tag="pt")
                    nc.tensor.transpose(tps[:, 0:64], hc[:, :], ident[:64, :64])
                    nc.any.tensor_copy(hnnat[:, b * L:(b + 1) * L], tps[:, 0:64])
                    # scores for this block
                    sc2 = pch.tile([128, 64], f32, tag="sc2")
                    nc.vector.tensor_mul(
                        sc2[:, :], hnnat[:, b * L:(b + 1) * L], p_bc[:, :]
                    )
                    nc.vector.tensor_reduce(
                        scorecol[:, b:b + 1], sc2[:, :], axis=AX.X, op=Alu.add
                    )

                # ---- LN scalars ----
                scol = pg.tile([64, 1], f32, tag="scol")
                nc.vector.tensor_reduce(scol[:, :], spart[:, :], axis=AX.X, op=Alu.add)
                sqcol = pg.tile([64, 1], f32, tag="sqcol")
                nc.vector.tensor_reduce(sqcol[:, :], sqpart[:, :], axis=AX.X, op=Alu.add)
                mu_ps = ppt.tile([128, 512], f32, tag="pt")
                nc.tensor.matmul(mu_ps[:1, 0:1], scol[:, :], ones64[:, :],
                                 start=True, stop=True)
                sq_ps = ppt.tile([128, 512], f32, tag="pt")
                nc.tensor.matmul(sq_ps[:1, 0:1], sqcol[:, :], ones64[:, :],
                                 start=True, stop=True)
                mu = pg.tile([1, 1], f32, tag="mu")
                nc.vector.tensor_scalar_mul(mu[:, :], mu_ps[:1, 0:1], 1.0 / (N * L))
                msq = pg.tile([1, 1], f32, tag="msq")
                nc.vector.tensor_scalar_mul(msq[:, :], sq_ps[:1, 0:1], 1.0 / (N * L))
                var = pg.tile([1, 1], f32, tag="var")
                nc.vector.tensor_mul(var[:, :], mu[:, :], mu[:, :])
                nc.vector.tensor_sub(var[:, :], msq[:, :], var[:, :])
                nc.vector.tensor_scalar_add(var[:, :], var[:, :], LN_EPS)
                sd = pg.tile([1, 1], f32, tag="sd")
                nc.scalar.activation(sd[:, :], var[:, :], Act.Sqrt)
                inv = pg.tile([1, 1], f32, tag="inv")
                nc.vector.reciprocal(inv[:, :], sd[:, :])
                muinv = pg.tile([1, 1], f32, tag="muinv")
                nc.vector.tensor_mul(muinv[:, :], mu[:, :], inv[:, :])
                # broadcast (inv, mu*inv) to all partitions via K=1 matmul
                scal2 = pg.tile([1, 2], f32, tag="scal2")
                nc.vector.tensor_copy(scal2[:, 0:1], inv[:, :])
                nc.vector.tensor_copy(scal2[:, 1:2], muinv[:, :])
                sb_ps = ppt.tile([128, 512], f32, tag="pt")
                nc.tensor.matmul(sb_ps[:, 0:2], ones_row[:, :], scal2[:, :],
                                 start=True, stop=True)
                scbc = pg.tile([128, 2], f32, tag="scbc")
                nc.any.tensor_copy(scbc[:, :], sb_ps[:, 0:2])

                # v = hn * (inv*rdeg) - mu*inv*rdeg  (per-row scalars)
                acol = pg.tile([128, NB], f32, tag="acol")
                nc.vector.tensor_scalar_mul(acol[:, :], rdeg[:, :], scbc[:, 0:1])
                bcol = pg.tile([128, NB], f32, tag="bcol")
                nc.vector.tensor_scalar(
                    bcol[:, :], rdeg[:, :], scbc[:, 1:2], -1.0,
                    op0=Alu.mult, op1=Alu.mult,
                )
                for b in range(NB):
                    nc.vector.tensor_scalar(
                        vnat[:, b * L:(b + 1) * L],
                        hnnat[:, b * L:(b + 1) * L],
                        acol[:, b:b + 1], bcol[:, b:b + 1],
                        op0=Alu.mult, op1=Alu.add,
                    )

                # ---- top-64 of scores ----
                v8 = pg.tile([128, 8], f32, tag="v8")
                nc.vector.max(out=v8[:, :], in_=scorecol[:, :])
                lidx = pg.tile([128, 8], u32, tag="lidx")
                nc.vector.max_index(out=lidx[:, :], in_max=v8[:, :], in_values=scorecol[:, :])
                lidxf = pg.tile([128, 8], f32, tag="lidxf")
                nc.vector.tensor_copy(lidxf[:, :], lidx[:, :])
                gidxf = pg.tile([128, 8], f32, tag="gidxf")
                nc.vector.tensor_scalar(
                    gidxf[:, :], lidxf[:, :], 128.0, iotacolf[:, :],
                    op0=Alu.mult, op1=Alu.add,
                )
                # candidates, transposed
                vt_ps = ppt.tile([128, 512], f32, tag="pt")
                nc.tensor.transpose(vt_ps[:8, 0:128], v8[:, :], ident[:, :])
                vts = pg.tile([8, 128], f32, tag="vts")
                nc.any.tensor_copy(vts[:, :], vt_ps[:8, 0:128])
                # flatten candidates to one row (DRAM bounce), then broadcast
                # them to all partitions via K=1 matmuls
                scr8 = pdr.tile([8, 128], f32, tag="scr8")
                nc.sync.dma_start(scr8[:, :], vts[:, :])
                flat = pg.tile([1, 1024], f32, tag="flat")
                nc.sync.dma_start(
                    flat[:, :], scr8.rearrange("l q -> (l q)").unsqueeze(0)
                )
                bc = ppbc.tile([128, 1024], f32, tag="bc")
                for h in range(2):
                    nc.tensor.matmul(
                        bc[:, h * 512:(h + 1) * 512], ones_row[:, :],
                        flat[:, h * 512:(h + 1) * 512],
                        start=True, stop=True,
                    )
                # rank among candidates (strict greater count)
                rankcol = pg.tile([128, 8], f32, tag="rankcol")
                for j in range(8):
                    nc.vector.tensor_scalar(
                        hnnat[:, 0:1024], bc[:, :], v8[:, j:j + 1], None,
                        op0=Alu.is_gt, op1=Alu.add,
                        accum_out=rankcol[:, j:j + 1],
                    )
                # scatter gidx by rank -> sorted index row [1, 64]
                si_ps = ppt.tile([128, 512], f32, tag="pt")
                m1 = pg.tile([128, 64], f32, tag="m1")
                for j in range(8):
                    nc.vector.tensor_scalar(
                        m1[:, :], iota64bc[:, :],
                        rankcol[:, j:j + 1], None, op0=Alu.is_equal,
                    )
                    nc.tensor.matmul(
                        si_ps[:1, 0:64], gidxf[:, j:j + 1], m1[:, :],
                        start=(j == 0), stop=(j == 7),
                    )
                sortedrow = pg.tile([1, 64], f32, tag="sortedrow")
                nc.any.tensor_copy(sortedrow[:, :], si_ps[:1, 0:64])
                # broadcast sorted indices to all partitions for Sel build
                se_ps = ppt.tile([128, 512], f32, tag="pt")
                nc.tensor.matmul(se_ps[:, 0:64], ones_row[:, :], sortedrow[:, :],
                                 start=True, stop=True)
                selin = pg.tile([128, 64], f32, tag="selin")
                nc.any.tensor_copy(selin[:, :], se_ps[:, 0:64])

                # ---- mm2: Z = (A @ v)^T, then agg natural ----
                z2 = ppb.tile([64, N], f32, tag="big")
                for kb in range(NB):
                    for c in range(N // 512):
                        nc.tensor.matmul(
                            z2[:, c * 512:(c + 1) * 512],
                            vnat[:, kb * L:(kb + 1) * L],
                            T[:, kb * N + c * 512: kb * N + (c + 1) * 512],
                            start=(kb == 0), stop=(kb == NB - 1),
                        )
                for b in range(NB):
                    zc = pch.tile([64, 128], f32, tag="zc")
                    nc.any.tensor_copy(zc[:, :], z2[:, b * 128:(b + 1) * 128])
                    tps = ppt.tile([128, 512], f32, tag="pt")
                    nc.tensor.transpose(tps[:, 0:64], zc[:, :], ident[:64, :64])
                    nc.any.tensor_copy(aggnat[:, b * L:(b + 1) * L], tps[:, 0:64])

                # ---- sub_reps^T = agg^T @ Sel ----
                st_ps = ppt.tile([128, 512], f32, tag="pt")
                for b in range(NB):
                    selb = pch.tile([128, 64], f32, tag="selb")
                    nc.vector.tensor_scalar(
                        selb[:, :], selin[:, :],
                        mvalcols[:, b:b + 1], None, op0=Alu.is_equal,
                    )
                    nc.tensor.matmul(
                        st_ps[:64, 0:64], aggnat[:, b * L:(b + 1) * L], selb[:, :],
                        start=(b == 0), stop=(b == NB - 1),
                    )
                subt = pg.tile([64, 64], f32, tag="subt")
                nc.any.tensor_copy(subt[:, :], st_ps[:64, 0:64])

                # ---- ops + conv + maxpool + alpha mix ----
                gmn = pg.tile([64, 1], f32, tag="gmn")
                nc.vector.tensor_scalar_mul(gmn[:, :], scol[:, :], scbc[0:64, 0:1])
                nc.vector.tensor_scalar(
                    gmn[:, :], gmn[:, :], 1.0 / N, scbc[0:64, 1:2],
                    op0=Alu.mult, op1=Alu.subtract,
                )
                op1 = pg.tile([64, 64], f32, tag="op1")
                t1 = pg.tile([64, 64], f32, tag="t1")
                nc.scalar.activation(op1[:, :], subt[:, :], Act.Relu)
                nc.vector.tensor_scalar_min(t1[:, :], subt[:, :], 0.0)
                nc.scalar.activation(t1[:, :], t1[:, :], Act.Exp)
                nc.vector.tensor_add(op1[:, :], op1[:, :], t1[:, :])
                nc.vector.tensor_scalar_add(op1[:, :], op1[:, :], -1.0)
                op2 = pg.tile([64, 64], f32, tag="op2")
                nc.vector.tensor_scalar_add(op2[:, :], subt[:, :], gmn[:, :])

                feats = pg.tile([64, 3], f32, tag="feats")
                for o, opt in enumerate((subt, op1, op2)):
                    cv_ps = ppt.tile([128, 512], f32, tag="pt")
                    nc.tensor.matmul(cv_ps[:64, 0:63], k0_sb[:, :], opt[:, 0:63],
                                     start=True, stop=False)
                    nc.tensor.matmul(cv_ps[:64, 0:63], k1_sb[:, :], opt[:, 1:64],
                                     start=False, stop=True)
                    crelu = pch.tile([64, 63], f32, tag="crelu")
                    nc.scalar.activation(crelu[:, :], cv_ps[:64, 0:63], Act.Relu,
                                         bias=cb_col[:, :])
                    nc.vector.tensor_reduce(feats[:, o:o + 1], crelu[:, :],
                                            axis=AX.X, op=Alu.max)

                fs = pg.tile([64, 3], f32, tag="fs")
                bcolg = pg.tile([64, 1], f32, tag="bcolg")
                nc.vector.tensor_mul(fs[:, :], feats[:, :], alph_bc[:, :])
                nc.vector.tensor_reduce(bcolg[:, :], fs[:, :], axis=AX.X, op=Alu.add)
                nc.sync.dma_start(b_out[gi].unsqueeze(-1), bcolg[:, :])

    return nc


_cache = {}


def _get_nc(n_graphs):
    if n_graphs not in _cache:
        _cache[n_graphs] = build_nc(n_graphs)
    return _cache[n_graphs]


def kernel(**inputs):
    g = np.ascontiguousarray(np.asarray(inputs["g"], dtype=np.float32))
    x = np.ascontiguousarray(np.asarray(inputs["x"], dtype=np.float32))
    original = np.asarray(inputs["original"], dtype=np.float32)
    w_disc = np.asarray(inputs["W_disc"], dtype=np.float32)
    shared = {
        "w_gcn": np.asarray(inputs["W_gcn"], dtype=np.float32),
        "b_gcn": np.asarray(inputs["b_gcn"], dtype=np.float32),
        "p_select": np.asarray(inputs["p_select"], dtype=np.float32),
        "conv_k": np.asarray(inputs["conv_k"], dtype=np.float32),
        "conv_b": np.asarray(inputs["conv_b"], dtype=np.float32),
        "log_alphas": np.asarray(inputs["log_alphas"], dtype=np.float32),
    }
    nc = _get_nc(GP)
    in_maps = []
    for c in range(NCORES):
        m = dict(shared)
        m["g4"] = g[c * GP:(c + 1) * GP]
        m["x4"] = x[c * GP:(c + 1) * GP]
        in_maps.append(m)
    res = run_bass_kernel_spmd(nc, in_maps, core_ids=list(range(NCORES)))
    B = np.concatenate([res.results[c]["b_out"] for c in range(NCORES)], axis=0)

    # discriminator tail (tiny): deterministic roll + bilinear scores
    shuf = np.roll(B, 1, axis=0)
    wc = original @ w_disc
    sc1 = np.sum(B * wc, axis=-1)
    sc2 = np.sum(shuf * wc, axis=-1)
    return np.concatenate([sc1, sc2]).astype(np.float32)
